# revision 25
# baseline (speedup 1.0000x reference)
"""DeepHit survival loss on 8 Trainium2 NeuronCores (Bass/Tile).

Math: the O(n^2) pairwise rank loss factorizes. With
  cdf[j,t]  = cumsum_t(exp(phi_j)) / sum(exp(phi_j))          (pad col folded in)
  E[j,t]    = exp(2*cdf[j,t])                                 (sigma = 0.5)
  W[j,d]    = 1{dur_j > d} + 1{dur_j == d}*(1 - ev_j) = 1{d <= dur_j - ev_j}
the pairwise sum  sum_ij rank_mat[i,j]*exp(-r_ij/sigma)  equals
  sum_i ev_i * exp(-2*cdf[i,lab_i]) * D[lab_i, dur_i],   D = E^T @ W  ([256,256]).

Sharding: batch rows n=8192 split as 1024 rows per core, 8 partition-tiles
of 128. Each core computes a partial D plus per-row cumsum-at-label and
row-sum stats; the host sums partial Ds, builds the u-weighted (lab,dur)
histogram P, takes <D,P>, and finishes the O(n) nll arithmetic.

Device schedule (vs the previous version):
- hazards are host-cast to bf16 and host-transposed to the exact SBUF
  layout [128, 8*258] (one contiguous 516B+ run per partition per chunk),
  halving the HBM stream to ~0.53MB/core.
- rows are HOST-SORTED by label within each core. Tile q then only holds
  labels in [32q-16, 32q+48), so the per-row cumsum-at-label shrinks from a
  256-wide masked reduce to a 64-wide band: cs[lab] = cs[band_lo-1] +
  sum_{band_lo<=t<=lab} exp(phi_t). The fixed column cs[band_lo-1] is a
  [128,1] copy; the band sum is a 64-wide scalar_tensor_tensor. (The host
  verifies the band invariant and falls back to a full-width variant if an
  adversarial input ever violates it.)
- E and W are bf16 (matmul at 1 cycle/row), scan/exp stats stay f32.
- D and pv leave the chip via kv_writeback descriptors PREPARED early on
  the Pool engine and FIRED by a single trigger_dma at the end: the
  SP-issue + HWDGE + DGE-delay chain (~1.9us) drops off the critical tail.
- no PE warmups; Act instruction order interleaves chunk-exps with E-exps
  to keep the activation engine (the bottleneck at ~5.6us) bubble-free.
"""

import os
import numpy as np

import concourse.bacc as bacc
import concourse.mybir as mybir
import concourse.tile as tile
from concourse import bass_utils

N, T = 8192, 256
TPP = T + 2                  # padded row length (sum col + scan-reset col)
N_CORES = 8
NLOC = N // N_CORES          # 1024 rows per core
NT = NLOC // 128             # 8 partition-tiles per core
ALPHA, SIGMA, EPS = 0.5, 0.5, 1e-7

CHUNKS = [1, 2, 3, 2]        # tiles per DMA/compute chunk
BAND = 64                    # label band width per tile (host-sorted rows)

f32 = mybir.dt.float32
bf16 = mybir.dt.bfloat16
i32 = mybir.dt.int32
Alu = mybir.AluOpType
Act = mybir.ActivationFunctionType

_CACHE = {}
LAST_RESULTS = None


def _blo(q):
    return max(0, 32 * q - 16)


def _build(full_band=False):
    nc = bacc.Bacc("TRN2", target_bir_lowering=False, debug=False)

    # cols 0:16 carry the 8 per-tile f32 label columns bit-packed as bf16
    # pairs (recovered on device via AP bitcast); tile q occupies cols
    # [16 + q*TPP, 16 + (q+1)*TPP)
    hazT_d = nc.dram_tensor("hazT", [128, 16 + NT * TPP], bf16,
                            kind="ExternalInput")
    # host-built W = 1{d <= dur-ev}, [128, q*T+d] layout, bf16
    W_d = nc.dram_tensor("W", [128, NT * T], bf16, kind="ExternalInput")

    D_d = nc.dram_tensor("D", [2, 128, T], f32, kind="ExternalOutput")
    # pv[0,:, 0:8]=band cum at label, [0,:,8:16]=2/sum_ng,
    # pv[1,:, 0:8]=cs[band_lo-1] (csfix; col 0 unused)
    pv_d = nc.dram_tensor("pv", [2, 128, 16], f32, kind="ExternalOutput")

    with tile.TileContext(nc) as tc:
        with (
            tc.tile_pool(name="const", bufs=1) as cpool,
            tc.tile_pool(name="work", bufs=2) as pool,
            tc.tile_pool(name="stage", bufs=1) as spool,
            tc.tile_pool(name="ps", bufs=1, space="PSUM") as pspool,
        ):
            # ---- Pool engine: constants ----
            iota_f = cpool.tile([128, 272], f32)
            nc.gpsimd.iota(iota_f[:], [[1, 272]], base=0, channel_multiplier=0,
                           allow_small_or_imprecise_dtypes=True)
            ctx0 = cpool.tile([128, 2], i32)
            nc.gpsimd.memset(ctx0[:], 0)

            CWMAX = max(CHUNKS) * TPP
            smask_t = cpool.tile([128, CWMAX], f32)
            smask3 = smask_t[:].rearrange("p (q t) -> p q t", q=max(CHUNKS))
            nc.gpsimd.memset(smask_t[:], 1.0)
            nc.gpsimd.memset(smask3[:, :, T : T + 1], 0.5)
            nc.gpsimd.memset(smask3[:, :, T + 1 : TPP], 0.0)

            # ---- SP: input DMAs. Order sets the serial DMA-engine stream:
            # haz chunks pace the Act pipeline, W halves slot in late enough
            # not to delay the exp chain but before their first matmul.
            W_all = spool.tile([128, NT * T], bf16)
            chunk_bufs = []
            q0 = 0
            for ci, csize in enumerate(CHUNKS):
                cw = csize * TPP + (16 if ci == 0 else 0)
                hazb = pool.tile([128, cw], bf16, tag=f"haz{ci}")
                lo = 0 if ci == 0 else 16 + q0 * TPP
                nc.sync.dma_start(hazb[:], hazT_d[:, lo : lo + cw])
                chunk_bufs.append(hazb)
                if ci == 2:
                    nc.sync.dma_start(W_all[:, 0 : 4 * T], W_d[:, 0 : 4 * T])
                q0 += csize
            nc.sync.dma_start(W_all[:, 4 * T : NT * T], W_d[:, 4 * T : NT * T])
            # the label columns ride in chunk 0; f32 view via bitcast
            dpk_t = chunk_bufs[0][:, 0:16].bitcast(f32)

            pv_t = spool.tile([128, 32], f32)
            D_sb = spool.tile([128, 2 * T], f32)
            D0_ps = pspool.tile([128, T], f32)
            D1_ps = pspool.tile([128, T], f32)

            # ---- pipeline over chunks ----
            exp_insts, e_insts = [], []
            q0 = 0
            for ci, csize in enumerate(CHUNKS):
                cw = csize * TPP
                hazb = chunk_bufs[ci]

                # exp(phi) batched per chunk; pad col gives exp(0)=1 (phi
                # max ~5 so no overflow; the gamma shift cancels in every
                # ratio used)
                expb = pool.tile([128, cw], f32, tag=f"expb{ci}", bufs=2)
                hoff = 16 if ci == 0 else 0
                exp_insts.append(nc.scalar.activation(
                    expb[:], hazb[:, hoff : hoff + cw], Act.Exp))

                # segmented prefix sum; op1 multiplies by the column mask:
                # 1.0 body, 0.5 at each sum column, 0.0 at each reset column
                csb = pool.tile([128, cw], f32, tag=f"cs{ci}", bufs=2)
                nc.vector.tensor_tensor_scan(
                    csb[:], expb[:], smask_t[:, 0:cw], 0.0, Alu.add, Alu.mult)
                cs3 = csb[:].rearrange("p (b t) -> p b t", b=csize)

                # rec = 2/sum_ng for the chunk's tiles, straight into pv
                nc.vector.reciprocal(
                    pv_t[:, 8 + q0 : 8 + q0 + csize], cs3[:, :, T : T + 1])

                for q2 in range(csize):
                    q = q0 + q2

                    # E = exp(cs * 2/sum_ng), scale fused into the activation
                    E_t = pool.tile([128, T], bf16, tag="E", bufs=4)
                    e_insts.append(nc.scalar.activation(
                        E_t[:], csb[:, q2 * TPP : q2 * TPP + T], Act.Exp,
                        scale=pv_t[:, 8 + q : 9 + q]))

                    # D += E^T @ W, t-chunked over two PSUM banks
                    nc.tensor.matmul(
                        D0_ps[:], E_t[:, 0:128], W_all[:, q * T : (q + 1) * T],
                        start=(q == 0), stop=(q == NT - 1))
                    nc.tensor.matmul(
                        D1_ps[:], E_t[:, 128:T], W_all[:, q * T : (q + 1) * T],
                        start=(q == 0), stop=(q == NT - 1))

                    # per-row cumsum at label
                    if full_band:
                        # tail = sum_{t>lab} exp (full width, is_gt); host
                        # derives cum = sum - tail
                        scr_t = pool.tile([128, T], f32, tag="scr")
                        nc.vector.scalar_tensor_tensor(
                            scr_t[:], iota_f[:, 0:T], dpk_t[:, q : q + 1],
                            expb[:, q2 * TPP : q2 * TPP + T],
                            Alu.is_gt, Alu.mult, accum_out=pv_t[:, q : q + 1])
                    else:
                        blo = _blo(q)
                        bw = min(BAND, TPP - blo)
                        if q > 0:
                            # csfix = cs[band_lo - 1]
                            nc.vector.tensor_copy(
                                pv_t[:, 16 + q : 17 + q],
                                csb[:, q2 * TPP + blo - 1 : q2 * TPP + blo])
                        # band cum = sum_{blo<=t<=lab} exp (iota values past
                        # 255 exceed any label, so pad cols are masked out)
                        scr_t = pool.tile([128, BAND], f32, tag="scr")
                        nc.vector.scalar_tensor_tensor(
                            scr_t[:, 0:bw], iota_f[:, blo : blo + bw],
                            dpk_t[:, q : q + 1],
                            expb[:, q2 * TPP + blo : q2 * TPP + blo + bw],
                            Alu.is_le, Alu.mult, accum_out=pv_t[:, q : q + 1])
                q0 += csize

            # D halves drain from PSUM through the Act engine into the
            # staging tile; the single trigger then fires both prepared
            # writebacks (pv + D) straight onto the DMA engines.
            # Pin the Act stream order (head-blocking FIFO engine): the
            # DMA-waiting chunk exps must not be scheduled ahead of already-
            # computable E exps. exp2 after E0, exp3 after E2 gives the
            # bubble-free interleave under the HWDGE-paced input stream.
            from concourse.instruction_name_ordered_set import (
                InstructionNameOrderedSet)
            for exp_i, e_i in ((2, 0), (3, 2)):
                deps = InstructionNameOrderedSet()
                deps.add(e_insts[e_i].ins.name)
                exp_insts[exp_i].ins.add_nosync_dependencies_from(deps)

            nc.scalar.copy(D_sb[:, 0:T], D0_ps[:])
            nc.vector.tensor_copy(D_sb[:, T : 2 * T], D1_ps[:])

            # Output descriptors: emitted after all pv/D_sb writers (so the
            # deferred RAW lands on the trigger, not as a WAR deadlock on the
            # writers), but the preps themselves execute early on Pool.
            dma_sem_pv = nc.alloc_semaphore("pv_dma_sem")
            dma_sem_d = nc.alloc_semaphore("d_dma_sem")
            nc.gpsimd.kv_writeback(
                pv_d[:].rearrange("b p (o t) -> b p o t", o=1),
                pv_t[:].rearrange("p (o b t) -> p o b t", o=1, b=2),
                ctx0[:], prepare_only=True, sem=dma_sem_pv)
            nc.gpsimd.kv_writeback(
                D_d[:].rearrange("b p (o t) -> b p o t", o=1),
                D_sb[:].rearrange("p (o b t) -> p o b t", o=1, b=2),
                ctx0[:], prepare_only=True, sem=dma_sem_d)
            nc.gpsimd.trigger_dma(count=None)

    _post_tile_surgery(nc, [dma_sem_pv, dma_sem_d])
    nc.compile()
    return nc


def _post_tile_surgery(nc, dma_sems):
    """Two sim/HW-consistent rewrites of the Tile-emitted sync graph.

    1. The kv_writeback PREPARE_ONLY instructions only write descriptors
       (addresses + ctx metadata); they need none of the staged DATA. Tile
       still hangs the staging writers' ticks on them, which serializes the
       ~1us Q7 desc-gen behind the last compute. Move those waits onto the
       trigger_dma (which is what actually starts the data read) so the
       preps run early on the idle Pool engine.
    2. The Tile teardown waits on the SWDGE lane sems (DMASW*), which on HW
       are bumped by the SWDGE ucode itself but never fire in cost-model
       simulation of a triggered transfer. Replace them with waits on the
       descriptor-baked private completion sems (the same SDMA-completion
       signal, +16 at transfer end), observable on both HW and sim.
    """
    fn = nc.m.functions[0]
    preps, trigger = [], None
    for blk in fn.blocks:
        for ins in blk.instructions:
            t = type(ins).__name__
            if t == "InstKVWritebackAnt" and getattr(ins, "gen_mode", 0) == 1:
                preps.append(ins)
            elif t == "InstTriggerDma":
                trigger = ins

    moved = []
    for p in preps:
        si = p.sync_info
        if si is None:
            continue
        moved.extend(list(si.on_wait))
        si.on_wait = []
    if trigger is not None and moved:
        si = trigger.sync_info
        cur = {(w.id): w for w in (list(si.on_wait) if si is not None else [])}
        for w in moved:
            old = cur.get(w.id)
            if old is None or (old.wait_value or 0) < (w.wait_value or 0):
                cur[w.id] = w
        si.on_wait = list(cur.values())

    # Map each DMASW lane wait to the private sems of the preps assigned to
    # that lane (round-robin in emission order, 16 ticks per prep): a wait
    # for "lane L >= 16*k" becomes waits on the first k lane-L preps' sems.
    lane_names = set()
    for blk in fn.blocks:
        for ins in blk.instructions:
            si = ins.sync_info
            if si is None:
                continue
            for w in si.on_wait:
                nm = w.ant_name or ""
                if nm.startswith("DMASW"):
                    lane_names.add(nm.split("_")[0])
    n_lanes = max(1, len(lane_names))
    lanes = {}
    for i, s in enumerate(dma_sems):
        lanes.setdefault(f"DMASW{i % n_lanes}", []).append(s)

    for blk in fn.blocks:
        for ins in blk.instructions:
            si = ins.sync_info
            if si is None:
                continue
            waits = list(si.on_wait)
            if not any((w.ant_name or "").startswith("DMASW") for w in waits):
                continue
            kept = [w for w in waits
                    if not (w.ant_name or "").startswith("DMASW")]
            have = {w.id for w in kept}
            for w in waits:
                nm = (w.ant_name or "")
                if not nm.startswith("DMASW"):
                    continue
                k = max(1, int(w.wait_value or 16) // 16)
                for s in lanes.get(nm.split("_")[0], [])[:k]:
                    if s.num not in have:
                        have.add(s.num)
                        kept.append(mybir.SyncWait(
                            sync_type="semaphore", id=s.num, ant_name=s.name,
                            wait_mode="sem-ge-imm", wait_value=16))
            si.on_wait = kept


def _get_nc(full_band=False):
    key = "full" if full_band else "band"
    if key not in _CACHE:
        _CACHE[key] = _build(full_band)
    return _CACHE[key]


def _check_band(lab_sorted):
    """lab_sorted: [NLOC] per-core sorted labels. True if every tile's
    labels fit its [blo, blo+BAND) band."""
    for q in range(NT):
        tl = lab_sorted[q * 128 : (q + 1) * 128]
        blo = _blo(q)
        if tl[0] < blo or tl[-1] >= blo + BAND:
            return False
    return True


def _make_in_maps(hazards, duration, event, label):
    import ml_dtypes

    dmef = (duration - event).astype(np.int64)
    labf = label.astype(np.float32)
    dgrid = np.arange(T, dtype=np.int64)[None, :]
    in_maps, perms, band_ok = [], [], True
    for c in range(N_CORES):
        sl = slice(c * NLOC, (c + 1) * NLOC)
        perm = np.argsort(label[sl], kind="stable")
        perms.append(perm)
        band_ok &= _check_band(label[sl][perm])

        hazp = np.zeros((NLOC, TPP), np.float32)
        hazp[:, 0:T] = hazards[sl][perm]
        # device layout: hazT[p, q*TPP + t] = row (q*128 + p) of the sorted
        # block -> [128, NT*TPP] with per-partition contiguous chunk runs
        hazT = np.empty((128, 16 + NT * TPP), dtype=ml_dtypes.bfloat16)
        hazT[:, 0:16] = np.ascontiguousarray(
            labf[sl][perm].reshape(NT, 128).T).view(np.uint16).view(
            ml_dtypes.bfloat16)
        hazT[:, 16:] = np.ascontiguousarray(
            hazp.reshape(NT, 128, TPP).transpose(1, 0, 2).reshape(128, NT * TPP)
        ).astype(ml_dtypes.bfloat16)

        Wc = (dgrid <= dmef[sl][perm][:, None])  # [NLOC, T] bool
        W = np.ascontiguousarray(
            Wc.reshape(NT, 128, T).transpose(1, 0, 2).reshape(128, NT * T)
        ).astype(ml_dtypes.bfloat16)
        in_maps.append({"hazT": hazT, "W": W})
    return in_maps, perms, band_ok


def _finish_host(hazards, duration, event, label, D_parts, pv_parts, perms,
                 full_band):
    """Host glue: O(n) + O(T^2) arithmetic from the per-core device outputs."""
    n = hazards.shape[0]
    dur = duration.astype(np.int64)
    ev = event.astype(np.int64)
    lab = label.astype(np.int64)

    cum_at_ng = np.empty(n, np.float64)
    sum_ng = np.empty(n, np.float64)
    D = np.zeros((T, T), np.float64)
    for c in range(N_CORES):
        Dc = D_parts[c].astype(np.float64)  # [2, 128, T]
        D += np.concatenate([Dc[0], Dc[1]], axis=0)

        pv = pv_parts[c].astype(np.float64)  # [2, 128, 16]
        rec = pv[0, :, 8:16].T.reshape(NLOC)          # [NLOC] sorted order
        s = 2.0 / rec
        if full_band:
            tail = pv[0, :, 0:8].T.reshape(NLOC)
            cum_s = s - tail
        else:
            csfix = pv[1, :, 0:8].T.reshape(NLOC)
            csfix[0:128] = 0.0                         # tile 0 has no fix col
            cum_s = csfix + pv[0, :, 0:8].T.reshape(NLOC)

        sl = slice(c * NLOC, (c + 1) * NLOC)
        inv = perms[c]
        cum_full = np.empty(NLOC, np.float64)
        sum_full = np.empty(NLOC, np.float64)
        cum_full[inv] = cum_s
        sum_full[inv] = s
        cum_at_ng[sl] = cum_full
        sum_ng[sl] = sum_full

    # rank loss: <D, P> with P the u-weighted (lab, dur) histogram
    cdf_at = cum_at_ng / sum_ng
    u = ev * np.exp(-2.0 * cdf_at)
    P = np.zeros((T, T), np.float64)
    np.add.at(P, (lab, dur), u)
    rank_loss = (D * P).sum() / (float(n) * float(n))

    # nll, following the reference formulas exactly
    gamma = np.maximum(hazards.max(axis=1), 0.0).astype(np.float64)
    eg = np.exp(-gamma)
    sum_ = sum_ng * eg
    cum_at = cum_at_ng * eg
    phi_at = hazards[np.arange(n), lab].astype(np.float64)
    evf = ev.astype(np.float64)
    part1 = (phi_at - gamma) * evf
    part2 = -np.log(np.maximum(sum_, 0.0) + EPS)
    part3 = np.log(np.maximum(sum_ - cum_at, 0.0) + EPS) * (1.0 - evf)
    nll = np.mean(-(part1 + part2 + part3))

    return np.float32(ALPHA * nll + (1.0 - ALPHA) * rank_loss)


def kernel(hazards, duration, event, label):
    global LAST_RESULTS
    hazards = np.asarray(hazards, dtype=np.float32)
    duration = np.asarray(duration)
    event = np.asarray(event)
    label = np.asarray(label)

    in_maps, perms, band_ok = _make_in_maps(hazards, duration, event, label)
    full_band = not band_ok
    nc = _get_nc(full_band)
    trace = bool(int(os.environ.get("KERNEL_TRACE", "0")))
    res = bass_utils.run_bass_kernel_spmd(
        nc,
        in_maps,
        core_ids=list(range(N_CORES)),
        trace=trace,
        trace_cores=list(range(N_CORES)) if trace else None,
        stitch_traces=False,
    )
    LAST_RESULTS = res
    D_parts = [r["D"] for r in res.results]
    pv_parts = [r["pv"] for r in res.results]
    return _finish_host(hazards, duration, event, label, D_parts, pv_parts,
                        perms, full_band)
